# revision 3
# baseline (speedup 1.0000x reference)
"""Trainium2 Bass kernel for the per-cluster Lorentz boost module.

Computation: out[b,c,i] = B[c,i,j] @ T[b,c,j] where the 4x4 boost matrix
B[c] is derived from a per-cluster boost 3-vector Bo[c].

Using the closed form of the boost matrix
    B = [[G0,          -D n^T        ],
         [-D n,  I + (g-1) n n^T     ]]
with  mag = clip(|Bo|, eps, 1-eps), n = Bo/mag, g = 1/sqrt(1-mag^2),
      D = g*mag, G0 = 1 + (g-1)*(n.n)
the apply factorizes to
    s     = n1*x1 + n2*x2 + n3*x3
    u     = (g-1)*s - D*x0
    out0  = G0*x0 - D*s
    out_i = x_i + n_i*u        (i = 1..3)

Sharding: pure data parallel over the batch axis (8192 -> 8 x 1024 rows).
Each core streams its (1024, 4096) f32 shard through SBUF in 128-row
tiles (batch on partitions, clusters*4 on the free dim -> fully
contiguous DMA), applies 17 fp32 tensor_tensor ops per tile using
stride-4 free-dim views, split across the Vector and GpSimd engines,
and streams the result back.  The tiny per-cluster coefficients are
precomputed on host and replicated across partitions once.
"""

import os
import sys

import numpy as np

_TRN_REPO = "/opt/trn_rl_repo"
if _TRN_REPO not in sys.path:
    sys.path.append(_TRN_REPO)

os.environ.setdefault("TRN_TYPE", "TRN2")

EPS = 1e-7

N_CORES = 8
B_FULL = 8192
B_CORE = B_FULL // N_CORES  # 1024 batch rows per core
C = 1024                    # clusters
FD = C * 4                  # free dim of a batch tile
N_TILES = B_CORE // 128     # 8 tiles of [128, 4096] per core


def _coef_from_bo(Bo: np.ndarray) -> np.ndarray:
    """Per-cluster coefficients, replicated across 128 partitions.

    Returns (128, 6*C) float32: [n1 | n2 | n3 | A=g-1 | D=g*mag | G0].
    """
    Bo = Bo.astype(np.float32)
    mag = np.sqrt(np.sum(np.square(Bo), axis=1, keepdims=True, dtype=np.float32))
    mag = np.clip(mag, np.float32(EPS), np.float32(1.0 - EPS)).astype(np.float32)
    n = (Bo / mag).astype(np.float32)
    g = (1.0 / np.sqrt(np.float32(1.0) - np.square(mag))).astype(np.float32)
    nn = np.sum(np.square(n), axis=1, keepdims=True, dtype=np.float32)
    A = (g - np.float32(1.0)).astype(np.float32)
    D = (g * mag).astype(np.float32)
    G0 = (np.float32(1.0) + A * nn).astype(np.float32)
    row = np.concatenate(
        [n[:, 0], n[:, 1], n[:, 2], A[:, 0], D[:, 0], G0[:, 0]]
    ).astype(np.float32)  # (6*C,)
    return np.ascontiguousarray(np.broadcast_to(row, (128, 6 * C)))


_CACHED = {}


def _build_program():
    """Build + compile the per-core Bass/Tile program (cached)."""
    if "nc" in _CACHED:
        return _CACHED["nc"]

    import concourse.bacc as bacc
    import concourse.mybir as mybir
    import concourse.tile as tile

    f32 = mybir.dt.float32
    mult = mybir.AluOpType.mult
    add = mybir.AluOpType.add
    subtract = mybir.AluOpType.subtract

    nc = bacc.Bacc("TRN2", target_bir_lowering=False, debug=False)

    t_in = nc.dram_tensor("t", [B_CORE, FD], f32, kind="ExternalInput").ap()
    coef_in = nc.dram_tensor("coef", [128, 6 * C], f32, kind="ExternalInput").ap()
    o_out = nc.dram_tensor("o", [B_CORE, FD], f32, kind="ExternalOutput").ap()

    t_tiled = t_in.rearrange("(n p) m -> n p m", p=128)
    o_tiled = o_out.rearrange("(n p) m -> n p m", p=128)

    with tile.TileContext(nc) as tc:
        with (
            tc.tile_pool(name="coefp", bufs=1) as coefp,
            tc.tile_pool(name="xp", bufs=2) as xp,
            tc.tile_pool(name="op", bufs=2) as op,
            tc.tile_pool(name="sp", bufs=2) as sp,
            tc.tile_pool(name="pp", bufs=2) as pp,
            tc.tile_pool(name="up", bufs=2) as up,
            tc.tile_pool(name="wp", bufs=2) as wp,
        ):
            ctile = coefp.tile([128, 6 * C], f32)
            nc.sync.dma_start(ctile[:], coef_in[:])
            cv = ctile[:].rearrange("p (k c) -> p k c", k=6)
            n1 = cv[:, 0]
            n2 = cv[:, 1]
            n3 = cv[:, 2]
            A = cv[:, 3]
            D = cv[:, 4]
            G0 = cv[:, 5]

            for it in range(N_TILES):
                X = xp.tile([128, FD], f32)
                nc.sync.dma_start(X[:], t_tiled[it])
                O = op.tile([128, FD], f32)

                xv = X[:].rearrange("p (c j) -> p j c", j=4)
                ov = O[:].rearrange("p (c j) -> p j c", j=4)
                x0, x1, x2, x3 = xv[:, 0], xv[:, 1], xv[:, 2], xv[:, 3]
                o0, o1, o2, o3 = ov[:, 0], ov[:, 1], ov[:, 2], ov[:, 3]

                s = sp.tile([128, C], f32)
                p2 = pp.tile([128, C], f32)
                p3 = pp.tile([128, C], f32)
                u = up.tile([128, C], f32)
                m2 = wp.tile([128, C], f32)
                m4 = wp.tile([128, C], f32)

                # s = n . x  (vector chain, with two products off-loaded)
                nc.vector.tensor_tensor(s[:], x1, n1, mult)
                nc.gpsimd.tensor_tensor(p2[:], x2, n2, mult)
                nc.vector.tensor_tensor(p3[:], x3, n3, mult)
                nc.vector.tensor_tensor(s[:], s[:], p2[:], add)
                nc.vector.tensor_tensor(s[:], s[:], p3[:], add)

                # independent products (ready at load time) on gpsimd
                nc.gpsimd.tensor_tensor(m2[:], x0, D, mult)   # D*x0
                nc.gpsimd.tensor_tensor(o0, x0, G0, mult)     # G0*x0

                # u = A*s - D*x0
                nc.vector.tensor_tensor(u[:], s[:], A, mult)
                nc.vector.tensor_tensor(u[:], u[:], m2[:], subtract)

                # out0 = G0*x0 - D*s
                nc.vector.tensor_tensor(m4[:], s[:], D, mult)
                nc.vector.tensor_tensor(o0, o0, m4[:], subtract)

                # out_i = x_i + n_i * u
                nc.vector.tensor_tensor(o1, u[:], n1, mult)
                nc.gpsimd.tensor_tensor(o2, u[:], n2, mult)
                nc.vector.tensor_tensor(o3, u[:], n3, mult)
                nc.gpsimd.tensor_tensor(o1, o1, x1, add)
                nc.gpsimd.tensor_tensor(o2, o2, x2, add)
                nc.vector.tensor_tensor(o3, o3, x3, add)

                nc.sync.dma_start(o_tiled[it], O[:])

    nc.compile()
    _CACHED["nc"] = nc
    return nc


def kernel(T: np.ndarray, Bo: np.ndarray) -> np.ndarray:
    from concourse.bass_utils import run_bass_kernel_spmd

    assert T.shape == (B_FULL, C, 4) and Bo.shape == (C, 3), (T.shape, Bo.shape)

    T = np.ascontiguousarray(T, dtype=np.float32)
    coef = _coef_from_bo(np.asarray(Bo))

    nc = _build_program()

    shards = T.reshape(N_CORES, B_CORE, FD)
    in_maps = [{"t": shards[i], "coef": coef} for i in range(N_CORES)]

    res = run_bass_kernel_spmd(nc, in_maps, core_ids=list(range(N_CORES)))

    out = np.empty((N_CORES, B_CORE, FD), dtype=np.float32)
    for i in range(N_CORES):
        out[i] = res.results[i]["o"]
    return out.reshape(B_FULL, C, 4)


# revision 4
# speedup vs baseline: 1.4320x; 1.4320x over previous
"""Trainium2 Bass kernel for the per-cluster Lorentz boost module.

out[b,c,i] = B[c,i,j] @ T[b,c,j], B derived from per-cluster boost
vectors Bo[c].  Boost matrix closed form:
    B = [[G0, -D n^T], [-D n, I + A n n^T]]
    mag = clip(|Bo|, eps, 1-eps), n = Bo/mag, g = 1/sqrt(1-mag^2)
    A = g-1, D = g*mag, G0 = 1 + A*(n.n)

Key algebraic trick (keeps every Vector-engine pass contiguous):
with lam = -D/A and s~ = lam*x0 + n.x,
    out_k = x_k + A*n_k*s~              (k = 1..3, exactly)
    out_0 = x0 + (-D)*s~ + eps*x0,  eps = (G0-1) + D*lam
so the whole apply is:
    P  = C1 * X          (C1 = [lam, n1, n2, n3] interleaved)
    s~ = segmented_reduce4(P)
    T  = A4 * bcast(s~)  (A4 = [-D, A n1, A n2, A n3] interleaved)
    e0 = eps * x0
    O  = X + T (+ e0 at slot 0)   -- identity-matmul accumulation on PE,
                                     evicted PSUM->SBUF by the Scalar engine
Engines: Vector does the 4 passes (3 full-rate contiguous + 1 strided),
PE does all adds, ACT evicts.  GpSimd is deliberately idle: its SW loops
contend with the Vector engine's SBUF ports (measured 2-3.6x slowdown).

Sharding: pure data parallel over batch (8192 -> 8 x 1024 rows per core).
"""

import os
import sys

import numpy as np

_TRN_REPO = "/opt/trn_rl_repo"
if _TRN_REPO not in sys.path:
    sys.path.append(_TRN_REPO)

os.environ.setdefault("TRN_TYPE", "TRN2")

EPS = 1e-7

N_CORES = 8
B_FULL = 8192
B_CORE = B_FULL // N_CORES  # 1024 batch rows per core
C = 1024                    # clusters
FD = C * 4                  # free dim of a batch tile
N_TILES = B_CORE // 128     # 8 tiles of [128, 4096] per core
N_CHUNK = FD // 512         # 8 psum chunks per tile


def _coef_from_bo(Bo: np.ndarray) -> dict:
    """Per-cluster coefficient arrays (float64 math, fp32 results)."""
    Bo = np.asarray(Bo, dtype=np.float32).astype(np.float64)
    mag = np.sqrt(np.sum(Bo * Bo, axis=1, keepdims=True))
    mag = np.clip(mag, EPS, 1.0 - EPS)
    n = Bo / mag                                   # (C,3)
    g = 1.0 / np.sqrt(1.0 - mag * mag)             # (C,1)
    A = g - 1.0
    D = g * mag
    nn = np.sum(n * n, axis=1, keepdims=True)
    G0 = 1.0 + A * nn
    lam32 = (-D / A).astype(np.float32)            # single fp32 rounding
    eps = ((G0 - 1.0) + D * lam32.astype(np.float64)).astype(np.float32)

    C1 = np.empty((C, 4), dtype=np.float32)
    C1[:, 0] = lam32[:, 0]
    C1[:, 1:] = n.astype(np.float32)
    A4 = np.empty((C, 4), dtype=np.float32)
    A4[:, 0] = (-D[:, 0]).astype(np.float32)
    A4[:, 1:] = (A * n).astype(np.float32)

    coef = np.concatenate(
        [
            C1.reshape(1, FD),
            A4.reshape(1, FD),
            eps.reshape(1, C),
        ],
        axis=1,
    )  # (1, 2*FD + C)
    coef_rep = np.ascontiguousarray(
        np.broadcast_to(coef, (128, coef.shape[1]))
    )
    return coef_rep


_CACHED = {}


def _build_program():
    if "nc" in _CACHED:
        return _CACHED["nc"]

    import concourse.bacc as bacc
    import concourse.mybir as mybir
    import concourse.tile as tile

    f32 = mybir.dt.float32
    mult = mybir.AluOpType.mult
    add = mybir.AluOpType.add

    nc = bacc.Bacc("TRN2", target_bir_lowering=False, debug=False)

    t_in = nc.dram_tensor("t", [B_CORE, FD], f32, kind="ExternalInput").ap()
    coef_in = nc.dram_tensor(
        "coef", [128, 2 * FD + C], f32, kind="ExternalInput"
    ).ap()
    ident_in = nc.dram_tensor("ident", [128, 128], f32, kind="ExternalInput").ap()
    o_out = nc.dram_tensor("o", [B_CORE, FD], f32, kind="ExternalOutput").ap()

    t_tiled = t_in.rearrange("(n p) m -> n p m", p=128)
    o_tiled = o_out.rearrange("(n p) m -> n p m", p=128)

    with tile.TileContext(nc) as tc:
        with (
            tc.tile_pool(name="coefp", bufs=1) as coefp,
            tc.tile_pool(name="xp", bufs=2) as xp,
            tc.tile_pool(name="pp", bufs=2) as pp,
            tc.tile_pool(name="tp", bufs=2) as tp,
            tc.tile_pool(name="op", bufs=2) as op,
            tc.tile_pool(name="sp", bufs=2) as sp,
            tc.tile_pool(name="ep", bufs=2) as ep,
            tc.tile_pool(name="psp", bufs=4, space="PSUM") as psp,
        ):
            ctile = coefp.tile([128, 2 * FD + C], f32)
            nc.sync.dma_start(ctile[:], coef_in[:])
            ident = coefp.tile([128, 128], f32)
            nc.sync.dma_start(ident[:], ident_in[:])

            C1 = ctile[:, 0:FD]
            A4 = ctile[:, FD : 2 * FD]
            EPSC = ctile[:, 2 * FD : 2 * FD + C]

            for it in range(N_TILES):
                X = xp.tile([128, FD], f32)
                nc.sync.dma_start(X[:], t_tiled[it])
                P = pp.tile([128, FD], f32)
                T = tp.tile([128, FD], f32)
                O = op.tile([128, FD], f32)
                s = sp.tile([128, C], f32)
                e0 = ep.tile([128, C], f32)

                x3 = X[:].rearrange("p (c j) -> p c j", j=4)
                xj = X[:].rearrange("p (c j) -> p j c", j=4)
                p3 = P[:].rearrange("p (c j) -> p c j", j=4)
                sb = s[:].rearrange("p (c one) -> p c one", one=1)

                # V1: P = C1 * X   (all products + lam*x0, contiguous)
                nc.vector.tensor_tensor(P[:], C1, X[:], mult)
                # V2: s~ = segmented sum over the 4 slots
                nc.vector.tensor_reduce(
                    s[:], p3[:, :, :], axis=mybir.AxisListType.X, op=add
                )
                # V3: T = A4 * bcast(s~)
                nc.vector.tensor_tensor(
                    T[:].rearrange("p (c j) -> p c j", j=4),
                    A4.rearrange("p (c j) -> p c j", j=4),
                    sb.broadcast_to([128, C, 4]),
                    mult,
                )
                # V4: e0 = eps * x0   (strided read, compact out)
                nc.vector.tensor_tensor(e0[:], EPSC, xj[:, 0], mult)

                # PE: O_psum = T + X (+ e0 at slot-0 columns); ACT evicts
                for k in range(N_CHUNK):
                    ps = psp.tile([128, 512], f32, tag="ps")
                    sl = slice(k * 512, (k + 1) * 512)
                    nc.tensor.matmul(
                        ps[:], ident[:], T[:, sl], start=True, stop=False
                    )
                    nc.tensor.matmul(
                        ps[:], ident[:], X[:, sl], start=False, stop=False
                    )
                    ps0 = ps[:].rearrange("p (c j) -> p j c", j=4)
                    nc.tensor.matmul(
                        ps0[:, 0, :],
                        ident[:],
                        e0[:, k * 128 : (k + 1) * 128],
                        start=False,
                        stop=True,
                    )
                    nc.scalar.copy(O[:, sl], ps[:])

                nc.sync.dma_start(o_tiled[it], O[:])

    nc.compile()
    _CACHED["nc"] = nc
    return nc


def kernel(T: np.ndarray, Bo: np.ndarray) -> np.ndarray:
    from concourse.bass_utils import run_bass_kernel_spmd

    assert T.shape == (B_FULL, C, 4) and Bo.shape == (C, 3), (T.shape, Bo.shape)

    T = np.ascontiguousarray(T, dtype=np.float32)
    coef = _coef_from_bo(Bo)
    ident = np.eye(128, dtype=np.float32)

    nc = _build_program()

    shards = T.reshape(N_CORES, B_CORE, FD)
    in_maps = [
        {"t": shards[i], "coef": coef, "ident": ident} for i in range(N_CORES)
    ]

    res = run_bass_kernel_spmd(nc, in_maps, core_ids=list(range(N_CORES)))

    out = np.empty((N_CORES, B_CORE, FD), dtype=np.float32)
    for i in range(N_CORES):
        out[i] = res.results[i]["o"]
    return out.reshape(B_FULL, C, 4)
